# revision 12
# baseline (speedup 1.0000x reference)
"""Trainium2 Bass kernel for nn_Candemann_Parafac_module_73993696575955.

Computes out = beta_0 + (8 * 0.2**3) * sum_{k, i>j} x[k, i, j] for
x of shape (7, 64, 64) float32 and scalar float32 beta_0.

The problem is tiny (114 KB in, scalar out), so sharding across cores is
counterproductive (any cross-core combine costs more than the whole kernel).
The same single-core program is replicated SPMD on cores 0-7 and core 0's
result is returned.

Host-side marshalling (layout only): the 14112 strict-lower-triangle
elements are packed 126-per-partition into partitions 0..111 of a
[113, 512]-byte buffer (full 512B DMA lines); partition 112 carries the
scalar beta_0 / CP_SUM so the single device-side scale folds it back to
beta_0. All arithmetic over x happens on device.

Device program (raw Bass, hand-placed semaphores):
  SP  : DMA xw -> SBUF (completion sem dsem gates compute); preload the
        output tensor's runtime base pointer into a register pair while
        the DMA is in flight; wait s2; register-load the 4-byte result
        from SBUF and TensorSave it straight to output DRAM (sequencer
        store - no DMA fixed costs on the output path); then clear the
        kernel semaphores for safe re-execution (SP's s2 wait is the
        program's last semaphore observation, so clearing here is
        race-free).
  DVE : wait dsem; tensor_scalar in0*CP_SUM with accum_out => col
        (per-partition sums; partition 112 becomes beta_0)      -> s1
  Pool: wait s1; partition_all_reduce(col) => res (cross-partition
        sum; final scalar in partition 0 of res)                -> s2

Both the Bass-init all-engine barrier and the Block-exit drain+barrier
are stripped: every cross-engine dependency is carried by the explicit
semaphores above, and run-to-run ordering is provided by the runtime's
own execution boundaries.
"""

import os

# request a core reset on runtime init — recovers a device left wedged by a
# previous (possibly unrelated) session; harmless when the device is healthy
os.environ.setdefault("NEURON_RT_RESET_CORES", "1")

import numpy as np

K = 7
N = 64
P = 113              # 112 data partitions + 1 beta partition
CPP = 126            # triangle elements per data partition (112 * 126 = 14112)
COLS = 128           # f32 slots per partition row (512 B DMA lines)
RB = COLS * 4        # 512 bytes per partition row
CP_SUM = float(np.float32(8 * 0.2**3))

N_CORES = 8

_CACHE = {}


def _strip_init_barrier(nc, mybir):
    fn = nc.m.functions[0]
    main_bb = fn.blocks[0]
    kept = [
        i
        for i in main_bb.instructions
        if not isinstance(i, (mybir.InstDrain, mybir.InstEventSemaphore))
    ]
    removed = len(main_bb.instructions) - len(kept)
    main_bb.instructions[:] = kept
    assert removed >= 10, f"expected to strip >=10 barrier insts, got {removed}"


def _strip_exit_barrier(nc, mybir):
    """Remove the Block-exit per-engine drains and the sem-only all-engine
    barrier. Explicit semaphores carry every cross-engine ordering edge, so
    the only thing the barrier still ordered was the semaphore clear - which
    now runs on SP behind the program's final semaphore wait."""
    removed = 0
    for bb in nc.m.functions[0].blocks:
        kept = []
        for i in bb.instructions:
            if isinstance(i, mybir.InstDrain) or (
                isinstance(i, mybir.InstEventSemaphore) and "barrier_" in str(i)
            ):
                removed += 1
            else:
                kept.append(i)
        bb.instructions[:] = kept
    assert removed >= 10, f"expected to strip >=10 exit insts, got {removed}"


def build_nc():
    import concourse.mybir as mybir
    import concourse.bass_isa as bass_isa
    from concourse import bacc

    nc = bacc.Bacc("TRN2", target_bir_lowering=False, debug=False)

    xw_d = nc.dram_tensor("xw", [P, RB], mybir.dt.uint8, kind="ExternalInput")
    o_d = nc.dram_tensor("out", [1, 64], mybir.dt.float32, kind="ExternalOutput")
    o_ptr = nc.pointer_tensor(o_d)

    _strip_init_barrier(nc, mybir)

    with (
        nc.sbuf_tensor("xw_sb", [P, RB], mybir.dt.uint8) as xw_sb,
        nc.sbuf_tensor("scratch", [P, CPP], mybir.dt.float32) as scratch,
        nc.sbuf_tensor("col", [P, 1], mybir.dt.float32) as col,
        nc.sbuf_tensor("res", [P, 1], mybir.dt.float32) as res,
        nc.semaphore("dsem") as dsem,
        nc.semaphore("s1") as s1,
        nc.semaphore("s2") as s2,
        nc.semaphore("s3") as s3,
        nc.Block(no_gpsimd_drain=True) as block,
    ):
        sem_ids = sorted(
            h.sem_id if hasattr(h, "sem_id") else h.num
            for h in (dsem, s1, s2, s3)
        )
        lo, hi = min(sem_ids), max(sem_ids)

        # only the 126 used columns feed the reduce; cols 126/127 are DMA
        # line padding (zeros) and would add DVE cycles for nothing
        x_v = xw_sb[:, 0 : CPP * 4].bitcast(mybir.dt.float32)

        @block.sync
        def _(sync):
            sync.dma_start(xw_sb[:, :], xw_d.ap()[:, :]).then_inc(dsem, 16)
            with sync.register64("oaddr") as addr, sync.register("rval") as r:
                # runtime-patched DRAM base of `out`; loadable while the
                # input DMA is still in flight
                sync.load(addr, o_ptr.ap()[0:1, 0:1].bitcast(mybir.dt.int32))
                sync.wait_ge(s2, 1)
                # the load executes only after the s2 wait — the program's
                # last semaphore observation — so s3 marks "all sems
                # consumed" and gates the Pool-side clear
                sync.load(r, res[0:1, 0:1].bitcast(mybir.dt.int32)).then_inc(
                    s3, 1
                )
                sync.store(addr, r)

        @block.vector
        def _(vector):
            vector.wait_ge(dsem, 16)
            vector.tensor_scalar(
                out=scratch[:],
                in0=x_v,
                scalar1=CP_SUM,
                scalar2=None,
                op0=mybir.AluOpType.mult,
                op1=mybir.AluOpType.add,
                accum_out=col[:],
            ).then_inc(s1, 1)

        @block.gpsimd
        def _(gpsimd):
            gpsimd.wait_ge(s1, 1)
            gpsimd.partition_all_reduce(
                res[:], col[:], channels=P, reduce_op=bass_isa.ReduceOp.add
            ).then_inc(s2, 1)
            # semaphore clear for safe re-execution, in parallel with SP's
            # result store; the wait rides on the clear itself
            gpsimd.sem_clear(range(lo, hi + 1))._wait_ge(s3, 1)

    _strip_exit_barrier(nc, mybir)

    nc.compile()
    return nc


def pack_inputs(x, beta_0):
    x = np.ascontiguousarray(np.asarray(x, dtype=np.float32)).reshape(-1)
    fin = _CACHE.get("perm")
    if fin is None:
        f = np.arange(K * N * N, dtype=np.int64)
        i = (f // N) % N
        j = f % N
        fin = f[i > j]
        _CACHE["perm"] = fin
    xw = np.zeros((P, COLS), dtype=np.float32)
    xw[0:112, 0:CPP] = x[fin].reshape(112, CPP)
    xw[112, 0] = np.float32(beta_0) / np.float32(CP_SUM)
    return {"xw": xw.view(np.uint8).reshape(P, RB)}


def _get_nc():
    if "nc" not in _CACHE:
        _CACHE["nc"] = build_nc()
    return _CACHE["nc"]


def _run(x, beta_0, **run_kwargs):
    from concourse.bass_utils import run_bass_kernel_spmd

    nc = _get_nc()
    in_map = pack_inputs(x, beta_0)
    return run_bass_kernel_spmd(
        nc, [in_map] * N_CORES, list(range(N_CORES)), **run_kwargs
    )


def kernel(x, beta_0):
    out = _run(x, beta_0)
    return np.float32(out.results[0]["out"][0, 0])


# revision 13
# speedup vs baseline: 1.0027x; 1.0027x over previous
"""Trainium2 Bass kernel for nn_Candemann_Parafac_module_73993696575955.

Computes out = beta_0 + (8 * 0.2**3) * sum_{k, i>j} x[k, i, j] for
x of shape (7, 64, 64) float32 and scalar float32 beta_0.

The problem is tiny (114 KB in, scalar out), so sharding across cores is
counterproductive (any cross-core combine costs more than the whole kernel).
The same single-core program is replicated SPMD on cores 0-7 and core 0's
result is returned.

Host-side marshalling (layout only): the 14112 strict-lower-triangle
elements are packed 126-per-partition into partitions 0..111 of a
[113, 512]-byte buffer (full 512B DMA lines); partition 112 carries the
scalar beta_0 / CP_SUM so the single device-side scale folds it back to
beta_0. All arithmetic over x happens on device.

Device program (raw Bass, hand-placed semaphores):
  SP  : DMA xw -> SBUF (completion sem dsem gates compute); preload the
        output tensor's runtime base pointer into a register pair while
        the DMA is in flight; wait s2; register-load the 4-byte result
        from SBUF and TensorSave it straight to output DRAM (sequencer
        store - no DMA fixed costs on the output path); then clear the
        kernel semaphores for safe re-execution (SP's s2 wait is the
        program's last semaphore observation, so clearing here is
        race-free).
  DVE : wait dsem; tensor_scalar in0*CP_SUM with accum_out => col
        (per-partition sums; partition 112 becomes beta_0)      -> s1
  Pool: wait s1; partition_all_reduce(col) => res (cross-partition
        sum; final scalar in partition 0 of res)                -> s2

Both the Bass-init all-engine barrier and the Block-exit drain+barrier
are stripped: every cross-engine dependency is carried by the explicit
semaphores above, and run-to-run ordering is provided by the runtime's
own execution boundaries.
"""

import os

# request a core reset on runtime init — recovers a device left wedged by a
# previous (possibly unrelated) session; harmless when the device is healthy
os.environ.setdefault("NEURON_RT_RESET_CORES", "1")

import numpy as np

K = 7
N = 64
P = 113              # 112 data partitions + 1 beta partition
CPP = 126            # triangle elements per data partition (112 * 126 = 14112)
COLS = 128           # f32 slots per partition row (512 B DMA lines)
RB = COLS * 4        # 512 bytes per partition row
CP_SUM = float(np.float32(8 * 0.2**3))

N_CORES = 8

_CACHE = {}


def _strip_init_barrier(nc, mybir):
    fn = nc.m.functions[0]
    main_bb = fn.blocks[0]
    kept = [
        i
        for i in main_bb.instructions
        if not isinstance(i, (mybir.InstDrain, mybir.InstEventSemaphore))
    ]
    removed = len(main_bb.instructions) - len(kept)
    main_bb.instructions[:] = kept
    assert removed >= 10, f"expected to strip >=10 barrier insts, got {removed}"


def _strip_exit_barrier(nc, mybir):
    """Remove the Block-exit per-engine drains and the sem-only all-engine
    barrier. Explicit semaphores carry every cross-engine ordering edge, so
    the only thing the barrier still ordered was the semaphore clear - which
    now runs on SP behind the program's final semaphore wait."""
    removed = 0
    for bb in nc.m.functions[0].blocks:
        kept = []
        for i in bb.instructions:
            if isinstance(i, mybir.InstDrain) or (
                isinstance(i, mybir.InstEventSemaphore) and "barrier_" in str(i)
            ):
                removed += 1
            else:
                kept.append(i)
        bb.instructions[:] = kept
    assert removed >= 10, f"expected to strip >=10 exit insts, got {removed}"


def build_nc():
    import concourse.mybir as mybir
    import concourse.bass_isa as bass_isa
    from concourse import bacc

    nc = bacc.Bacc("TRN2", target_bir_lowering=False, debug=False)

    xw_d = nc.dram_tensor("xw", [P, RB], mybir.dt.uint8, kind="ExternalInput")
    o_d = nc.dram_tensor("out", [1, 64], mybir.dt.float32, kind="ExternalOutput")
    o_ptr = nc.pointer_tensor(o_d)

    _strip_init_barrier(nc, mybir)

    with (
        nc.sbuf_tensor("xw_sb", [P, RB], mybir.dt.uint8) as xw_sb,
        nc.sbuf_tensor("scratch", [P, CPP], mybir.dt.float32) as scratch,
        nc.sbuf_tensor("col", [P, 1], mybir.dt.float32) as col,
        nc.sbuf_tensor("res", [P, 1], mybir.dt.float32) as res,
        nc.semaphore("dsem") as dsem,
        nc.semaphore("s1") as s1,
        nc.semaphore("s2") as s2,
        nc.semaphore("s3") as s3,
        nc.Block(no_gpsimd_drain=True) as block,
    ):
        sem_ids = sorted(
            h.sem_id if hasattr(h, "sem_id") else h.num
            for h in (dsem, s1, s2, s3)
        )
        lo, hi = min(sem_ids), max(sem_ids)

        # only the 126 used columns feed the reduce; cols 126/127 are DMA
        # line padding (zeros) and would add DVE cycles for nothing
        x_v = xw_sb[:, 0 : CPP * 4].bitcast(mybir.dt.float32)

        @block.sync
        def _(sync):
            sync.dma_start(xw_sb[:, :], xw_d.ap()[:, :]).then_inc(dsem, 16)
            with sync.register64("oaddr") as addr, sync.register("rval") as r:
                # runtime-patched DRAM base of `out`; loadable while the
                # input DMA is still in flight
                sync.load(addr, o_ptr.ap()[0:1, 0:1].bitcast(mybir.dt.int32))
                sync.wait_ge(s2, 1)
                # the load executes only after the s2 wait — the program's
                # last semaphore observation — so s3 marks "all sems
                # consumed" and gates the Pool-side clear
                sync.load(r, res[0:1, 0:1].bitcast(mybir.dt.int32)).then_inc(
                    s3, 1
                )
                sync.store(addr, r)

        @block.vector
        def _(vector):
            vector.wait_ge(dsem, 16)
            vector.tensor_scalar(
                out=scratch[:],
                in0=x_v,
                scalar1=CP_SUM,
                scalar2=None,
                op0=mybir.AluOpType.mult,
                op1=mybir.AluOpType.add,
                accum_out=col[:],
            ).then_inc(s1, 1)

        @block.gpsimd
        def _(gpsimd):
            gpsimd.wait_ge(s1, 1)
            gpsimd.partition_all_reduce(
                res[:], col[:], channels=P, reduce_op=bass_isa.ReduceOp.add
            ).then_inc(s2, 1)

        @block.scalar
        def _(act):
            # semaphore clear for safe re-execution, in parallel with SP's
            # result store; Act has the cheapest decode+recv of the idle
            # sequencers, so its clear finishes before SP's store does
            act.sem_clear(range(lo, hi + 1))._wait_ge(s3, 1)

    _strip_exit_barrier(nc, mybir)

    nc.compile()
    return nc


def pack_inputs(x, beta_0):
    x = np.ascontiguousarray(np.asarray(x, dtype=np.float32)).reshape(-1)
    fin = _CACHE.get("perm")
    if fin is None:
        f = np.arange(K * N * N, dtype=np.int64)
        i = (f // N) % N
        j = f % N
        fin = f[i > j]
        _CACHE["perm"] = fin
    xw = np.zeros((P, COLS), dtype=np.float32)
    xw[0:112, 0:CPP] = x[fin].reshape(112, CPP)
    xw[112, 0] = np.float32(beta_0) / np.float32(CP_SUM)
    return {"xw": xw.view(np.uint8).reshape(P, RB)}


def _get_nc():
    if "nc" not in _CACHE:
        _CACHE["nc"] = build_nc()
    return _CACHE["nc"]


def _run(x, beta_0, **run_kwargs):
    from concourse.bass_utils import run_bass_kernel_spmd

    nc = _get_nc()
    in_map = pack_inputs(x, beta_0)
    return run_bass_kernel_spmd(
        nc, [in_map] * N_CORES, list(range(N_CORES)), **run_kwargs
    )


def kernel(x, beta_0):
    out = _run(x, beta_0)
    return np.float32(out.results[0]["out"][0, 0])


# revision 18
# speedup vs baseline: 1.0031x; 1.0003x over previous
"""Trainium2 Bass kernel for nn_Candemann_Parafac_module_73993696575955.

Computes out = beta_0 + (8 * 0.2**3) * sum_{k, i>j} x[k, i, j] for
x of shape (7, 64, 64) float32 and scalar float32 beta_0.

The problem is tiny (114 KB in, scalar out), so sharding across cores is
counterproductive (any cross-core combine costs more than the whole kernel).
The same single-core program is replicated SPMD on cores 0-7 and core 0's
result is returned.

Host-side marshalling (layout only): the 14112 strict-lower-triangle
elements are packed row-major into partitions 0..110 of a [112, 512]-byte
buffer (full 512B DMA lines, zero padded); partition 111 carries the
scalar beta_0 / CP_SUM so the single device-side scale folds it back to
beta_0. All arithmetic over x happens on device.

Device program (raw Bass, hand-placed semaphores):
  SP  : DMA xw -> SBUF (completion sem dsem gates compute); preload the
        output tensor's runtime base pointer into a register pair while
        the DMA is in flight; wait s2; register-load the 4-byte result
        from SBUF and TensorSave it straight to output DRAM (sequencer
        store - no DMA fixed costs on the output path); then clear the
        kernel semaphores for safe re-execution (SP's s2 wait is the
        program's last semaphore observation, so clearing here is
        race-free).
  DVE : wait dsem; tensor_scalar in0*CP_SUM with accum_out => col
        (per-partition sums; partition 112 becomes beta_0)      -> s1
  Pool: wait s1; partition_all_reduce(col) => res (cross-partition
        sum; final scalar in partition 0 of res)                -> s2

Both the Bass-init all-engine barrier and the Block-exit drain+barrier
are stripped: every cross-engine dependency is carried by the explicit
semaphores above, and run-to-run ordering is provided by the runtime's
own execution boundaries.
"""

import os

# request a core reset on runtime init — recovers a device left wedged by a
# previous (possibly unrelated) session; harmless when the device is healthy
os.environ.setdefault("NEURON_RT_RESET_CORES", "1")

import numpy as np

K = 7
N = 64
P = 112              # 111 data partitions + 1 beta partition
NTRI = 14112         # strict-lower-triangle element count
COLS = 128           # f32 slots per partition row (512 B DMA lines)
RB = COLS * 4        # 512 bytes per partition row; 111*128=14208 >= 14112
CP_SUM = float(np.float32(8 * 0.2**3))

N_CORES = 8

_CACHE = {}


def _strip_init_barrier(nc, mybir):
    fn = nc.m.functions[0]
    main_bb = fn.blocks[0]
    kept = [
        i
        for i in main_bb.instructions
        if not isinstance(i, (mybir.InstDrain, mybir.InstEventSemaphore))
    ]
    removed = len(main_bb.instructions) - len(kept)
    main_bb.instructions[:] = kept
    assert removed >= 10, f"expected to strip >=10 barrier insts, got {removed}"


def _strip_exit_barrier(nc, mybir):
    """Remove the Block-exit per-engine drains and the sem-only all-engine
    barrier. Explicit semaphores carry every cross-engine ordering edge, so
    the only thing the barrier still ordered was the semaphore clear - which
    now runs on SP behind the program's final semaphore wait."""
    removed = 0
    for bb in nc.m.functions[0].blocks:
        kept = []
        for i in bb.instructions:
            if isinstance(i, mybir.InstDrain) or (
                isinstance(i, mybir.InstEventSemaphore) and "barrier_" in str(i)
            ):
                removed += 1
            else:
                kept.append(i)
        bb.instructions[:] = kept
    assert removed >= 10, f"expected to strip >=10 exit insts, got {removed}"


def build_nc():
    import concourse.mybir as mybir
    import concourse.bass_isa as bass_isa
    from concourse import bacc

    nc = bacc.Bacc("TRN2", target_bir_lowering=False, debug=False)

    xw_d = nc.dram_tensor("xw", [P, RB], mybir.dt.uint8, kind="ExternalInput")
    o_d = nc.dram_tensor("out", [1, 64], mybir.dt.float32, kind="ExternalOutput")
    o_ptr = nc.pointer_tensor(o_d)

    _strip_init_barrier(nc, mybir)

    with (
        nc.sbuf_tensor("xw_sb", [P, RB], mybir.dt.uint8) as xw_sb,
        nc.sbuf_tensor("scratch", [P, COLS], mybir.dt.float32) as scratch,
        nc.sbuf_tensor("col", [P, 1], mybir.dt.float32) as col,
        nc.sbuf_tensor("res", [P, 1], mybir.dt.float32) as res,
        nc.semaphore("dsem") as dsem,
        nc.semaphore("s1") as s1,
        nc.semaphore("s2") as s2,
        nc.semaphore("s3") as s3,
        nc.Block(no_gpsimd_drain=True) as block,
    ):
        sem_ids = sorted(
            h.sem_id if hasattr(h, "sem_id") else h.num
            for h in (dsem, s1, s2, s3)
        )
        lo, hi = min(sem_ids), max(sem_ids)

        x_v = xw_sb[:, 0:RB].bitcast(mybir.dt.float32)

        @block.sync
        def _(sync):
            sync.dma_start(xw_sb[:, :], xw_d.ap()[:, :]).then_inc(dsem, 16)
            with sync.register64("oaddr") as addr, sync.register("rval") as r:
                # runtime-patched DRAM base of `out`; loadable while the
                # input DMA is still in flight
                sync.load(addr, o_ptr.ap()[0:1, 0:1].bitcast(mybir.dt.int32))
                sync.wait_ge(s2, 1)
                # the load executes only after the s2 wait — the program's
                # last semaphore observation — so s3 marks "all sems
                # consumed" and gates the Pool-side clear
                sync.load(r, res[0:1, 0:1].bitcast(mybir.dt.int32)).then_inc(
                    s3, 1
                )
                sync.store(addr, r)

        @block.vector
        def _(vector):
            vector.wait_ge(dsem, 16)
            vector.tensor_scalar(
                out=scratch[:],
                in0=x_v,
                scalar1=CP_SUM,
                scalar2=None,
                op0=mybir.AluOpType.mult,
                op1=mybir.AluOpType.add,
                accum_out=col[:],
            ).then_inc(s1, 1)

        @block.gpsimd
        def _(gpsimd):
            gpsimd.wait_ge(s1, 1)
            gpsimd.partition_all_reduce(
                res[:], col[:], channels=P, reduce_op=bass_isa.ReduceOp.add
            ).then_inc(s2, 1)

        @block.scalar
        def _(act):
            # semaphore clear for safe re-execution, in parallel with SP's
            # result store; Act has the cheapest decode+recv of the idle
            # sequencers, so its clear finishes before SP's store does
            act.sem_clear(range(lo, hi + 1))._wait_ge(s3, 1)

    _strip_exit_barrier(nc, mybir)

    nc.compile()
    return nc


def pack_inputs(x, beta_0):
    x = np.ascontiguousarray(np.asarray(x, dtype=np.float32)).reshape(-1)
    fin = _CACHE.get("perm")
    if fin is None:
        f = np.arange(K * N * N, dtype=np.int64)
        i = (f // N) % N
        j = f % N
        fin = f[i > j]
        _CACHE["perm"] = fin
    xw = np.zeros((P, COLS), dtype=np.float32)
    xw.reshape(-1)[0:NTRI] = x[fin]
    xw[P - 1, 0] = np.float32(beta_0) / np.float32(CP_SUM)
    return {"xw": xw.view(np.uint8).reshape(P, RB)}


def _get_nc():
    if "nc" not in _CACHE:
        _CACHE["nc"] = build_nc()
    return _CACHE["nc"]


def _run(x, beta_0, **run_kwargs):
    from concourse.bass_utils import run_bass_kernel_spmd

    nc = _get_nc()
    in_map = pack_inputs(x, beta_0)
    return run_bass_kernel_spmd(
        nc, [in_map] * N_CORES, list(range(N_CORES)), **run_kwargs
    )


def kernel(x, beta_0):
    out = _run(x, beta_0)
    return np.float32(out.results[0]["out"][0, 0])
